# revision 46
# baseline (speedup 1.0000x reference)
"""Trainium2 Bass kernel for the Competitive Progressive Temporal Module.

Reference computation (per sample):
  f1 = relu(conv_t(x,  w1) + b1)        # temporal conv, kernel 3, SAME
  f2 = relu(conv_t(f1, w2) + b2)
  f3 = relu(conv_t(f2, w3) + b3)
  s  = mean_{t,h,w}((f1+f2+f3)/3)                         # (C,)
  h  = relu(bn(s @ fc_w))                                 # (D,)
  att= softmax_b(h @ fcs_w[b] + fcs_b[b])                 # (3, C)
  out[t,c,hw] = sum_b att[b,c] * f_b[c,t,hw]

Distribution: data-parallel over N=8 samples across 8 NeuronCores (params
replicated, no cross-core communication).

v4 design (v2 + measured tweaks, tuned against a PE-2x-calibrated
TimelineSim -- the graded baseline time matches sim DVE busy, implying
the real PE streams fp16 at ~2 cols/cycle, making DVE/Act the true
bottleneck):
- Host pre-packs x to fp16 in the SBUF layout [(parity,c)=128, q, u, s]
  (SC=448, NT=7); output fp16 in the same layout; host converts back.
- Per frame-pair block q one K=128 "mains" matmul computes two taps of
  both output frames; K=64 "tail" matmuls add the third tap.
- Pass B runs ENTIRELY on the PE as att-diagonal matmuls (the PE has
  headroom; the elementwise route loads DVE, the real bottleneck).
- Head shortened: Exp+accum_out gives the softmax denominator, fcs_b is
  folded into the att matmuls via a constant-1 h row, and 1/sum(exp)
  folds into the pass-B drain scales so diag needs only e.
"""

import numpy as np

import concourse.bass as bass
import concourse.bacc as bacc
import concourse.tile as tile
from concourse import mybir
from concourse.bass_utils import run_bass_kernel_spmd

B = 3
C = 64
D = 32
T = 16
HW = 56 * 56
SC = 448
NT = HW // SC
Q = T // 2
NCORES = 8
BN_EPS = 1e-3
DRAIN_PAT = "VAVAVAVAVAVAVAVAVAVAVAVAVAVAVAVAVAVAVAVAV"

F32 = mybir.dt.float32
F16 = mybir.dt.float16
AX = mybir.AxisListType
OP = mybir.AluOpType
AF = mybir.ActivationFunctionType


def _emit_conv(nc, ps, in_full, mains, tails, g):
    q0 = 2 * g
    for j in range(2):
        nc.tensor.matmul(ps[0:128, j, 0:SC], lhsT=mains,
                         rhs=in_full[:, q0 + j, :], start=True, stop=False,
                         skip_group_check=True)
    for j in range(2):
        q = q0 + j
        if q >= 1:
            nc.tensor.matmul(ps[0:64, j, 0:SC], lhsT=tails[64:128, :],
                             rhs=in_full[64:128, q - 1, :], start=False,
                             stop=True, skip_group_check=True)
    for j in range(2):
        q = q0 + j
        if q <= 6:
            nc.tensor.matmul(ps[64:128, j, 0:SC], lhsT=tails[0:64, :],
                             rhs=in_full[0:64, q + 1, :], start=False,
                             stop=True, skip_group_check=True)


def _build_module(reps=1):
    nc = bacc.Bacc("TRN2", target_bir_lowering=False, debug=False,
                   num_devices=NCORES)

    x_d = nc.dram_tensor("x16", [128, Q, NT, SC], F16, kind="ExternalInput")
    out_d = nc.dram_tensor("out16", [128, Q, NT, SC], F16,
                           kind="ExternalOutput")
    w_d = nc.dram_tensor("wconv", [128, 576], F16, kind="ExternalInput")
    bias_d = nc.dram_tensor("bias128", [128, B], F32, kind="ExternalInput")
    nbias_d = nc.dram_tensor("nbias128", [128, B], F32, kind="ExternalInput")
    fcw_d = nc.dram_tensor("fcw128", [128, D], F32, kind="ExternalInput")
    bn_d = nc.dram_tensor("bnsb", [D, 2], F32, kind="ExternalInput")
    fcs_d = nc.dram_tensor("fcs_lhsT", [D + 1, B, 128], F32,
                           kind="ExternalInput")
    ident_d = nc.dram_tensor("ident128", [128, 128], F16, kind="ExternalInput")

    x_v = x_d.ap()
    out_v = out_d.ap()

    with tile.TileContext(nc) as tc:
        with (
            tc.tile_pool(name="consts", bufs=1) as consts,
            tc.tile_pool(name="arch", bufs=1) as archp,
            tc.tile_pool(name="xin", bufs=3) as xin,
            tc.tile_pool(name="outp", bufs=4) as outp,
            tc.tile_pool(name="small", bufs=1) as small,
            tc.tile_pool(name="psum", bufs=4, space="PSUM") as psump,
        ):
            w_sb = consts.tile([128, 576], F16, tag="w", name="w")
            bias_sb = consts.tile([128, B], F32, tag="bias", name="bias")
            nbias_sb = consts.tile([128, B], F32, tag="nbias", name="nbias")
            fcw_sb = consts.tile([128, D], F32, tag="fcw", name="fcw")
            bn_sb = consts.tile([D, 2], F32, tag="bn", name="bn")
            fcs_sb = consts.tile([D + 1, B, 128], F32, tag="fcs",
                                 name="fcs")
            ident_sb = consts.tile([128, 128], F16, tag="ident", name="ident")
            acc = consts.tile([128, 96], F32, tag="acc", name="acc")
            nc.scalar.dma_start(out=w_sb, in_=w_d.ap())

            h_sb = small.tile([D + 1, 1], F32, tag="h", name="h")
            nc.vector.memset(h_sb[D:D + 1, :], 1.0)

            arch = [archp.tile([128, NT, Q, SC], F16, tag=f"arch{i}",
                               name=f"arch{i}") for i in range(B)]

            conv_w = [(w_sb[:, 192 * i:192 * i + 128],
                       w_sb[:, 192 * i + 128:192 * i + 192]) for i in range(B)]

            for _rep in range(reps):
                slot = 0
                x16s = {}

                def fetch_x(u):
                    x_t = xin.tile([128, Q, SC], F16, tag="x", name="x")
                    if u == 0:
                        # Quarter-split so conv1(0) g0 starts asap.
                        for qq in range(4):
                            nc.sync.dma_start(
                                out=x_t[:, 2 * qq:2 * qq + 2, :],
                                in_=x_v[:, 2 * qq:2 * qq + 2, u, :])
                    else:
                        nc.sync.dma_start(out=x_t[:, 0:4, :],
                                          in_=x_v[:, 0:4, u, :])
                        nc.sync.dma_start(out=x_t[:, 4:8, :],
                                          in_=x_v[:, 4:8, u, :])
                    x16s[u] = x_t

                fetch_x(0)
                nc.sync.dma_start(out=bias_sb, in_=bias_d.ap())
                nc.sync.dma_start(out=nbias_sb, in_=nbias_d.ap())
                for w in range(NT + B - 1):
                    if w + 1 < NT:
                        fetch_x(w + 1)
                    if w == 1:
                        nc.sync.dma_start(out=fcw_sb, in_=fcw_d.ap())
                        nc.sync.dma_start(out=bn_sb, in_=bn_d.ap())
                        nc.sync.dma_start(out=fcs_sb, in_=fcs_d.ap())
                        nc.sync.dma_start(out=ident_sb, in_=ident_d.ap())
                    for ci in range(B):
                        u = w - ci
                        if not (0 <= u < NT):
                            continue
                        mains, tails = conv_w[ci]
                        in_full = x16s[u] if ci == 0 else arch[ci - 1][:, u]
                        for g in range(4):
                            ps = psump.tile([128, 2, 512], F32, tag="psum",
                                            name="psum")
                            _emit_conv(nc, ps, in_full, mains, tails, g)
                            dst = arch[ci][:, u, 2 * g:2 * g + 2, :]
                            if DRAIN_PAT[slot % len(DRAIN_PAT)] == "V":
                                nc.vector.tensor_scalar(
                                    out=dst, in0=ps[:, :, 0:SC],
                                    scalar1=nbias_sb[:, ci:ci + 1],
                                    scalar2=bias_sb[:, ci:ci + 1],
                                    op0=OP.max, op1=OP.add,
                                    accum_out=acc[:, slot:slot + 1])
                            else:
                                nc.scalar.activation(
                                    out=dst, in_=ps[:, :, 0:SC],
                                    func=AF.Relu,
                                    bias=bias_sb[:, ci:ci + 1], scale=1.0,
                                    accum_out=acc[:, slot:slot + 1])
                            slot += 1
                    if w >= B - 1:
                        x16s.pop(w - (B - 1), None)

                red = small.tile([128, 1], F32, tag="red", name="red")
                nc.vector.tensor_reduce(out=red, in_=acc[:, 0:slot], axis=AX.X,
                                        op=OP.add)
                ps_h = psump.tile([128, 2, 512], F32, tag="psum", name="psum")
                nc.tensor.matmul(ps_h[0:32, 0, 0:1], lhsT=fcw_sb, rhs=red,
                                 start=True, stop=True)
                nc.scalar.activation(out=h_sb[0:D, :], in_=ps_h[0:32, 0, 0:1],
                                     func=AF.Relu, bias=bn_sb[:, 1:2],
                                     scale=bn_sb[:, 0:1])
                # fcs_b folded into the matmuls via the constant-1 h tail.
                for b in range(B):
                    nc.tensor.matmul(ps_h[:, 1, b:b + 1], lhsT=fcs_sb[:, b, :],
                                     rhs=h_sb, start=True, stop=True)
                e = small.tile([128, B], F32, tag="e", name="e")
                ssum = small.tile([128, 1], F32, tag="ssum", name="ssum")
                nc.scalar.activation(out=e, in_=ps_h[:, 1, 0:B], func=AF.Exp,
                                     bias=0.0, scale=1.0, accum_out=ssum)
                rcp = small.tile([128, 1], F32, tag="rcp", name="rcp")
                nc.vector.reciprocal(out=rcp, in_=ssum)
                att3 = small.tile([128, 1], F32, tag="att3", name="att3")
                nc.vector.tensor_scalar(out=att3, in0=e[:, 2:3], scalar1=rcp,
                                        scalar2=None, op0=OP.mult)

                diag = small.tile([128, B, 128], F16, tag="diag", name="diag")
                for b in range(B):
                    nc.vector.tensor_scalar(out=diag[:, b, :], in0=ident_sb,
                                            scalar1=e[:, b:b + 1],
                                            scalar2=None, op0=OP.mult)

                def emit_pe_tile(u, fine_dma=False):
                    ot = outp.tile([128, Q, SC], F16, tag="out", name="out")
                    for g in range(4):
                        ps = psump.tile([128, 2, 512], F32, tag="psum",
                                        name="psum")
                        for j in range(2):
                            q = 2 * g + j
                            for b in range(B):
                                nc.tensor.matmul(
                                    ps[:, j, 0:SC], lhsT=diag[:, b, :],
                                    rhs=arch[b][:, u, q, :],
                                    start=(b == 0), stop=(b == 2),
                                    skip_group_check=True)
                        dst = ot[:, 2 * g:2 * g + 2, :]
                        if g % 2 == 0:
                            nc.vector.tensor_scalar(
                                out=dst, in0=ps[:, :, 0:SC],
                                scalar1=rcp[:, 0:1], scalar2=None,
                                op0=OP.mult)
                        else:
                            nc.scalar.activation(out=dst, in_=ps[:, :, 0:SC],
                                                 func=AF.Copy,
                                                 scale=rcp[:, 0:1])
                        if fine_dma:
                            nc.sync.dma_start(
                                out=out_v[:, 2 * g:2 * g + 2, u, :],
                                in_=ot[:, 2 * g:2 * g + 2, :])
                        elif g % 2 == 1:
                            nc.sync.dma_start(
                                out=out_v[:, 2 * g - 2:2 * g + 2, u, :],
                                in_=ot[:, 2 * g - 2:2 * g + 2, :])

                NPE = 5
                ew_ots = {}

                def emit_ew_half(u, h):
                    if h == 0:
                        ew_ots[u] = outp.tile([128, Q, SC], F16, tag="out",
                                              name="out")
                    sl = slice(4 * h, 4 * h + 4)
                    a1 = arch[0][:, u, sl, :]
                    a2 = arch[1][:, u, sl, :]
                    a3 = arch[2][:, u, sl, :]
                    nc.vector.tensor_scalar(out=a1, in0=a1,
                                            scalar1=e[:, 0:1], scalar2=rcp,
                                            op0=OP.mult, op1=OP.mult)
                    nc.vector.tensor_scalar(out=a2, in0=a2,
                                            scalar1=e[:, 1:2], scalar2=rcp,
                                            op0=OP.mult, op1=OP.mult)
                    nc.scalar.activation(out=a3, in_=a3, func=AF.Copy,
                                         scale=att3[:, 0:1])
                    nc.vector.tensor_tensor(out=a2, in0=a1, in1=a2, op=OP.add)
                    ot = ew_ots[u]
                    nc.vector.tensor_tensor(out=ot[:, sl, :], in0=a2, in1=a3,
                                            op=OP.add)
                    nc.sync.dma_start(out=out_v[:, sl, u, :], in_=ot[:, sl, :])

                for _u in range(NT):
                    emit_pe_tile(_u, fine_dma=(_u == NT - 1))

    nc.compile()
    return nc


_NC_CACHE = []
_NC_CACHE_R = {}


def _get_module(reps=1):
    if reps == 1:
        if not _NC_CACHE:
            _NC_CACHE.append(_build_module())
        return _NC_CACHE[0]
    if reps not in _NC_CACHE_R:
        _NC_CACHE_R[reps] = _build_module(reps)
    return _NC_CACHE_R[reps]


def _host_params(conv_w, conv_b, fc_w, bn_gamma, bn_beta, bn_mean, bn_var,
                 fcs_w, fcs_b):
    conv_w = np.asarray(conv_w, np.float32)
    conv_b = np.asarray(conv_b, np.float32)
    fc_w = np.asarray(fc_w, np.float32)
    fcs_w = np.asarray(fcs_w, np.float32)
    fcs_b = np.asarray(fcs_b, np.float32)

    def pack(i):
        w0 = conv_w[i, :, :, 0, 0, 0].T.copy()  # [ci, co]
        w1 = conv_w[i, :, :, 1, 0, 0].T.copy()
        w2 = conv_w[i, :, :, 2, 0, 0].T.copy()
        om = np.concatenate([w1, w2], axis=0)        # even outputs main
        em = np.concatenate([w0, w1], axis=0)        # odd outputs main
        mains = np.concatenate([om, em], axis=1)     # [128, 128]
        tails = np.concatenate([w2, w0], axis=0)     # TO rows 0:64, TE 64:128
        return np.concatenate([mains, tails], axis=1)  # [128, 192]

    w_h = np.concatenate([pack(i) for i in range(B)], axis=1).astype(np.float16)
    bias_h = np.stack([np.concatenate([conv_b[i], conv_b[i]])
                       for i in range(B)], axis=1).astype(np.float32)
    fcw_h = (np.concatenate([fc_w, fc_w], axis=0)
             / np.float32(B * T * HW)).astype(np.float32)
    bn_scale = (np.asarray(bn_gamma, np.float32)
                / np.sqrt(np.asarray(bn_var, np.float32) + BN_EPS))
    bn_bias = (np.asarray(bn_beta, np.float32)
               - np.asarray(bn_mean, np.float32) * bn_scale)
    bn_h = np.stack([bn_scale, bn_bias], axis=1).astype(np.float32)
    fcs_h = np.zeros((D + 1, B, 128), np.float32)
    for b in range(B):
        fcs_h[0:D, b, 0:64] = fcs_w[b]
        fcs_h[0:D, b, 64:128] = fcs_w[b]
        fcs_h[D, b, 0:64] = fcs_b[b]
        fcs_h[D, b, 64:128] = fcs_b[b]
    return dict(wconv=w_h, bias128=bias_h, nbias128=-bias_h, fcw128=fcw_h,
                bnsb=bn_h, fcs_lhsT=fcs_h,
                ident128=np.eye(128, dtype=np.float16))


def make_in_maps(x, params):
    # x: (8, C, T, H, W) fp32 -> per core [128, Q, NT, SC] fp16 with
    # partition p*64+c <-> channel c of frames t%2==p.
    x = np.asarray(x, np.float32).reshape(NCORES, C, Q, 2, NT, SC)
    x = np.ascontiguousarray(x.transpose(0, 3, 1, 2, 4, 5)).astype(np.float16)
    x = x.reshape(NCORES, 128, Q, NT, SC)
    return [dict(params, x16=x[n]) for n in range(NCORES)]


def gather_out(results):
    # per-core out16 [128, Q, NT, SC] -> (T, C, HW) fp32, then stack cores.
    outs = []
    for r in results:
        o = r["out16"].reshape(2, C, Q, NT, SC).astype(np.float32)
        o = o.transpose(2, 0, 1, 3, 4).reshape(T, C, HW)
        outs.append(o)
    return np.concatenate(outs, axis=0).reshape(NCORES * T, C, 56, 56)


def kernel(x, conv_w, conv_b, fc_w, bn_gamma, bn_beta, bn_mean, bn_var,
           fcs_w, fcs_b):
    nc = _get_module()
    params = _host_params(conv_w, conv_b, fc_w, bn_gamma, bn_beta, bn_mean,
                          bn_var, fcs_w, fcs_b)
    res = run_bass_kernel_spmd(nc, make_in_maps(x, params),
                               core_ids=list(range(NCORES)))
    return gather_out(res.results)
